# revision 17
# baseline (speedup 1.0000x reference)
"""DenseKVMemory Trainium2 kernel.

Reference op (B=1, M=32768, S=512, H=8, D=64, fp32):
  - clear memory at doc boundaries (start_of_sequence)
  - joint attention over [memory (unmasked), segment (strict causal)]
  - scatter-write segment K/V into the ring buffer at write_index

Distribution: memory rows are sharded evenly across the 8 cores (4096
rows/core, all heads).  Each core computes the unnormalized softmax
partials (numerator [H,64,S] and denominator [H,S], fused via a
ones-column on the value matmul) over its shard for all heads, plus the
causal segment attention for ONE head (head index == core index; the
program is identical on every core, only the per-core input data
differs).  The ring-buffer write is a DRAM->DRAM copy of the
host-prescattered shard.  The host combines partials (gather +
normalize only).

Score matmuls run transposed (scoresT[t, q]) so that both GEMM operands
use their natural / host-pretransposed layouts and no on-chip
transposes are needed:
  scoresT = kT_tile.T @ qT          (fp16, contraction over D=64)
  p       = exp(scoresT)            (ScalarE, PSUM -> SBUF fp16)
  y_acc[q-chunk] += p_chunk.T @ [v_tile | 1]   (contraction over t=128)
The PV matmul keeps the LARGE operand (p, full 128 columns, fp16) as
the stationary side so the weight port's fast-weight-load (2 cols/cyc,
overlapped with the previous matmul's stream) absorbs it, and streams
only 65 columns of [v | ones]; column 64 of the accumulator is the
softmax denominator.  The PE clock on these parts is pinned at 1.2 GHz,
so minimizing total streamed matmul columns is what matters.
"""

import os

import numpy as np

os.environ.setdefault("MYCRO_LOCAL_CACHE", "1")

B, M, S, H, D = 1, 32768, 512, 8, 64
NCORES = 8
MC = M // NCORES  # memory rows per core
PAIRS = H // 2
NEG = np.float32(-1e30)
SCALE = np.float32(D) ** np.float32(-0.5)

_BUILD_CACHE = {}
QK_F16 = True


def _group_sizes(mt):
    """Split mt m-tiles into groups alternating between the two PSUM
    score slots (slot A: 4 banks, slot B: 3 banks; + accum = 8 banks).
    An EVEN number of groups per head keeps the A/B alternation intact
    across head boundaries so consecutive groups never share a slot
    (a shared slot serializes QK(g+1) behind exp(g) on the PE)."""
    gs = []
    rem = mt
    while rem >= 7:
        gs += [4, 3]
        rem -= 7
    if rem == 4:
        gs += [2, 2]
    elif rem == 3:
        gs += [2, 1]
    elif rem:
        gs += [rem]
    assert sum(gs) == mt
    return gs


def build_nc(mc=MC, s=S, h=H, d=D, use_f32r=True):
    """Build + compile the single-core Bass program (same on all cores)."""
    import concourse.bacc as bacc
    import concourse.mybir as mybir
    import concourse.tile as tile

    f32 = mybir.dt.float32
    bf16 = mybir.dt.float16
    mm_dt = mybir.dt.float16 if QK_F16 else (
        mybir.dt.float32r if use_f32r else f32)
    pairs = h // 2
    mt = mc // 128
    gsizes = _group_sizes(mt)
    st = s // 128  # segment m-tiles
    assert st <= 4

    nc = bacc.Bacc("TRN2", target_bir_lowering=False, debug=False,
                   num_devices=NCORES)

    kT_d = nc.dram_tensor("kt_sh", [pairs, 128, mc], mm_dt, kind="ExternalInput")
    vaug_d = nc.dram_tensor("memv_aug", [mc, h, d + 1], bf16, kind="ExternalInput")
    kscat_d = nc.dram_tensor("memk_scat", [mc, h, d], f32, kind="ExternalInput")
    vscat_d = nc.dram_tensor("memv_scat", [mc, h, d], f32, kind="ExternalInput")
    qT_d = nc.dram_tensor("qt_pack", [pairs, 128, s], mm_dt, kind="ExternalInput")
    ktseg_d = nc.dram_tensor("kt_seg", [d, s], mm_dt, kind="ExternalInput")
    qtseg_d = nc.dram_tensor("qt_seg", [d, s], mm_dt, kind="ExternalInput")
    vseg_d = nc.dram_tensor("v_seg", [s, d + 1], bf16, kind="ExternalInput")
    mask_d = nc.dram_tensor("maskt", [s, s], f32, kind="ExternalInput")

    nmk_d = nc.dram_tensor("new_memk", [mc, h, d], f32, kind="ExternalOutput")
    nmv_d = nc.dram_tensor("new_memv", [mc, h, d], f32, kind="ExternalOutput")
    part_d = nc.dram_tensor("mem_part", [h, 128, s // 128, d + 1], f32,
                            kind="ExternalOutput")
    seg_d = nc.dram_tensor("seg_part", [128, s // 128, d + 1], f32,
                           kind="ExternalOutput")

    Exp = mybir.ActivationFunctionType.Exp

    # DRAM->DRAM scatter copy chunks, paced across the kernel so they
    # never monopolize the DMA rings / HBM ahead of compute.
    d2d = []
    crows = max(128, mc // 32)
    for src, dst in ((kscat_d, nmk_d), (vscat_d, nmv_d)):
        for r0 in range(0, mc, crows):
            r1 = min(mc, r0 + crows)
            d2d.append((dst[r0:r1], src[r0:r1]))

    # per-group metadata for the software pipeline: QK(g)+exp(g) are
    # emitted one group AHEAD of PV(g-1) so the PE never waits on exp.
    # Score PSUM slot alternates by global parity (A: 4 banks, B: 3).
    groups = []
    for head in range(h):
        mt0 = 0
        for gs in gsizes:
            groups.append(dict(head=head, mt0=mt0, gs=gs, seg=False))
            mt0 += gs
    groups.append(dict(head=h, mt0=0, gs=st, seg=True))
    prev_slot = 1
    for g in groups:
        slot = 1 - prev_slot
        if g["gs"] > (4, 3)[slot]:
            slot = 0
        g["slot"] = slot
        prev_slot = slot
    d2d_every = max(1, len(groups) // (len(d2d) + 1))

    with tile.TileContext(nc) as tc:
        with (
            tc.tile_pool(name="big", bufs=1) as big,
            tc.tile_pool(name="vp", bufs=7) as vp,
            tc.tile_pool(name="pp", bufs=3) as pp,
            tc.tile_pool(name="op", bufs=2) as op,
            tc.tile_pool(name="ps", bufs=1, space="PSUM") as ps,
        ):
            # queries first (every QK reads them), then kT chunks drip-fed
            qt = big.tile([128, pairs, s], mm_dt, tag="qt")
            for p in range(pairs):
                nc.sync.dma_start(out=qt[:, p, :], in_=qT_d[p])

            nchunk = (mt + 3) // 4
            kt = [[None] * nchunk for _ in range(pairs)]
            kt_todo = [(p, c) for p in range(pairs) for c in range(nchunk)]

            def drip_kt(n):
                for _ in range(n):
                    if not kt_todo:
                        return
                    p, c = kt_todo.pop(0)
                    c0, c1 = c * 512, min(mc, (c + 1) * 512)
                    t = big.tile([128, c1 - c0], mm_dt, tag=f"kt{p}_{c}",
                                 name=f"kt{p}_{c}")
                    nc.sync.dma_start(out=t[:], in_=kT_d[p, :, c0:c1])
                    kt[p][c] = t

            seg_tiles = {}

            def load_seg():
                maskt = big.tile([128, st, s], f32, tag="mask", name="maskt")
                nc.sync.dma_start(
                    out=maskt[:],
                    in_=mask_d.rearrange("(j p) q -> p j q", p=128))
                ktseg = big.tile([d, s], mm_dt, tag="ktseg", name="ktseg")
                nc.sync.dma_start(out=ktseg[:], in_=ktseg_d[:])
                qtseg = big.tile([d, s], mm_dt, tag="qtseg", name="qtseg")
                nc.sync.dma_start(out=qtseg[:], in_=qtseg_d[:])
                vseg = big.tile([128, st, d + 1], bf16, tag="vseg",
                                name="vseg")
                nc.sync.dma_start(
                    out=vseg[:],
                    in_=vseg_d.rearrange("(j p) f -> p j f", p=128))
                seg_tiles.update(maskt=maskt, ktseg=ktseg, qtseg=qtseg,
                                 vseg=vseg)

            accs = {}
            n_pv = {}

            def emit_vt(g):
                """Prefetch the value tile for group g."""
                if g["seg"]:
                    g["vt"] = seg_tiles["vseg"]
                    return
                gs, mt0, head = g["gs"], g["mt0"], g["head"]
                vt = vp.tile([128, gs, d + 1], bf16, tag="vt", name="vt")
                nc.sync.dma_start(
                    out=vt[:],
                    in_=vaug_d[mt0 * 128:(mt0 + gs) * 128, head, :]
                    .rearrange("(j p) f -> p j f", p=128))
                g["vt"] = vt

            def emit_qk_exp(g):
                """QK matmuls + (mask) + exp for group g."""
                gs, mt0 = g["gs"], g["mt0"]
                cap = (4, 3)[g["slot"]]
                sc = ps.tile([128, cap, s], f32, tag=f"sc{g['slot']}",
                             name=f"sc{g['slot']}")
                pt = pp.tile([128, cap, s], bf16, tag=f"pt{g['slot']}",
                             name=f"pt{g['slot']}")
                if g["seg"]:
                    for j in range(gs):
                        t = mt0 + j
                        nc.tensor.matmul(
                            sc[:, j, :],
                            seg_tiles["ktseg"][:, t * 128:(t + 1) * 128],
                            seg_tiles["qtseg"][:], start=True, stop=True)
                    nc.vector.tensor_add(
                        sc[:, :gs, :], sc[:, :gs, :],
                        seg_tiles["maskt"][:, mt0:mt0 + gs, :])
                else:
                    head = g["head"]
                    pair, half = head // 2, head % 2
                    p0 = 64 * half
                    for j in range(gs):
                        m = mt0 + j
                        nc.tensor.matmul(
                            sc[:, j, :],
                            kt[pair][m // 4][p0:p0 + d,
                                             (m % 4) * 128:(m % 4) * 128 + 128],
                            qt[p0:p0 + d, pair, :],
                            start=True, stop=True)
                nc.scalar.activation(pt[:, :gs, :], sc[:, :gs, :], Exp)
                g["pt"] = pt

            nq = s // 128

            def emit_pv(g):
                """PV accumulation for group g (+ spill when head done).
                Flipped orientation: p chunk is the stationary operand
                (bf16 full-128-col -> FWL), v streams (65 cols/tile)."""
                head, gs = g["head"], g["gs"]
                last_n = st if g["seg"] else mt
                if head not in accs:
                    accs[head] = ps.tile([128, nq, d + 1], f32, tag="acc",
                                         name=f"acc{head}")
                    n_pv[head] = 0
                acc = accs[head]
                voff = g["mt0"] if g["seg"] else 0
                for j in range(gs):
                    for qc in range(nq):
                        # start only on the head's very first matmul:
                        # start=True marks the whole 2KB PSUM bank as
                        # pending-zero, which covers all nq accumulators.
                        nc.tensor.matmul(
                            acc[:, qc, :],
                            g["pt"][:, j, qc * 128:(qc + 1) * 128],
                            g["vt"][:, voff + j, :],
                            start=(n_pv[head] == 0 and qc == 0),
                            stop=(n_pv[head] == last_n - 1),
                            skip_group_check=True)
                    n_pv[head] += 1
                if n_pv[head] == last_n:
                    yt = op.tile([128, nq, d + 1], f32, tag="yt", name="yt")
                    nc.vector.tensor_copy(yt[:], acc[:])
                    dst = seg_d[:] if g["seg"] else part_d[head]
                    nc.sync.dma_start(out=dst, in_=yt[:])
                    del accs[head]

            ng = len(groups)
            seg_load_at = max(0, ng - 8)
            if seg_load_at == 0:
                load_seg()
            prev = None
            nd2d = 0
            for gi, g in enumerate(groups):
                if gi == 0:
                    for gg in groups[:4]:
                        if not gg["seg"]:
                            emit_vt(gg)
                elif gi + 3 < ng and not groups[gi + 3]["seg"]:
                    emit_vt(groups[gi + 3])
                drip_kt(2)
                if gi == seg_load_at and gi > 0:
                    load_seg()
                if g["seg"]:
                    emit_vt(g)
                emit_qk_exp(g)
                if prev is not None:
                    emit_pv(prev)
                prev = g
                if gi % d2d_every == d2d_every - 1 and nd2d < len(d2d):
                    dst, src = d2d[nd2d]
                    nc.sync.dma_start(out=dst, in_=src)
                    nd2d += 1
            emit_pv(prev)
            drip_kt(len(kt_todo))
            while nd2d < len(d2d):
                dst, src = d2d[nd2d]
                nc.sync.dma_start(out=dst, in_=src)
                nd2d += 1

    nc.compile()
    return nc


def _get_nc():
    key = (MC, S, H, D)
    if key not in _BUILD_CACHE:
        _BUILD_CACHE[key] = build_nc()
    return _BUILD_CACHE[key]


def make_in_maps(mem_keys, mem_vals, keys, values, queries,
                 start_of_sequence, write_index,
                 mc=MC, ncores=NCORES):
    """Host-side input marshaling: clearing, prescatter, shard, transpose."""
    mem_keys = np.asarray(mem_keys, np.float32)
    mem_vals = np.asarray(mem_vals, np.float32)
    keys = np.asarray(keys, np.float32)
    values = np.asarray(values, np.float32)
    queries = np.asarray(queries, np.float32)
    m = mem_keys.shape[1]
    s, h, d = keys.shape[1:]
    pairs = h // 2

    keep = not bool(np.asarray(start_of_sequence).reshape(-1)[0])
    mk = mem_keys[0] if keep else np.zeros_like(mem_keys[0])
    mv = mem_vals[0] if keep else np.zeros_like(mem_vals[0])
    k_seg, v_seg = keys[0], values[0]
    q = queries[0] * np.float32(d) ** np.float32(-0.5)

    wi = int(np.asarray(write_index))
    wi_c = max(0, min(wi, m - s))
    k_scat = mk.copy()
    k_scat[wi_c:wi_c + s] = k_seg
    v_scat = mv.copy()
    v_scat[wi_c:wi_c + s] = v_seg

    bf16 = np.float16
    kq_dt = np.float16 if QK_F16 else np.float32

    tri = np.arange(s, dtype=np.int32)
    maskT = np.where(tri[:, None] < tri[None, :],
                     np.float32(0.0), NEG).astype(np.float32)
    qT = np.ascontiguousarray(q.transpose(1, 2, 0).reshape(pairs, 2 * d, s))
    ones_mc = np.ones((mc, h, 1), np.float32)
    ones_s = np.ones((s, 1), np.float32)

    in_maps = []
    for c in range(ncores):
        r0, r1 = c * mc, (c + 1) * mc
        in_maps.append({
            "kt_sh": np.ascontiguousarray(
                mk[r0:r1].transpose(1, 2, 0)
                .reshape(pairs, 2 * d, mc)).astype(kq_dt),
            "memv_aug": np.concatenate(
                [mv[r0:r1], ones_mc], axis=2).astype(bf16),
            "memk_scat": np.ascontiguousarray(k_scat[r0:r1]),
            "memv_scat": np.ascontiguousarray(v_scat[r0:r1]),
            "qt_pack": qT.astype(kq_dt),
            "kt_seg": np.ascontiguousarray(k_seg[:, c % h, :].T).astype(kq_dt),
            "qt_seg": np.ascontiguousarray(q[:, c % h, :].T).astype(kq_dt),
            "v_seg": np.concatenate(
                [v_seg[:, c % h, :], ones_s], axis=1).astype(bf16),
            "maskt": maskT,
        })
    return in_maps


def combine(results, m, s, h, d, wi, ncores=NCORES):
    """Gather per-core partials into the final outputs."""
    new_mk = np.concatenate(
        [results[c]["new_memk"] for c in range(ncores)], axis=0)[None]
    new_mv = np.concatenate(
        [results[c]["new_memv"] for c in range(ncores)], axis=0)[None]
    tot = np.zeros((h, 128, s // 128, d + 1), np.float32)
    for c in range(ncores):
        tot += results[c]["mem_part"]
    for head in range(h):
        tot[head] += results[head % ncores]["seg_part"]
    # tot[h, r, qc, :]: query index = qc*128 + r
    y = (tot[..., :d] / tot[..., d:d + 1])  # [h, r, qc, d]
    y = y.transpose(2, 1, 0, 3).reshape(s, h, d)[None]
    new_wi = np.int32((wi + s) % m)
    return (np.ascontiguousarray(y, dtype=np.float32), new_mk, new_mv, new_wi)


def kernel(mem_keys, mem_vals, keys, values, queries,
           start_of_sequence, write_index):
    from concourse.bass_utils import run_bass_kernel_spmd

    in_maps = make_in_maps(mem_keys, mem_vals, keys, values, queries,
                           start_of_sequence, write_index)
    nc = _get_nc()
    res = run_bass_kernel_spmd(nc, in_maps, core_ids=list(range(NCORES)))
    kernel._last_results = res
    wi = int(np.asarray(write_index))
    return combine(res.results, M, S, H, D, wi)


# revision 20
# speedup vs baseline: 1.2585x; 1.2585x over previous
"""DenseKVMemory Trainium2 kernel.

Reference op (B=1, M=32768, S=512, H=8, D=64, fp32):
  - clear memory at doc boundaries (start_of_sequence)
  - joint attention over [memory (unmasked), segment (strict causal)]
  - scatter-write segment K/V into the ring buffer at write_index

Distribution: memory rows are sharded evenly across the 8 cores (4096
rows/core, all heads).  Each core computes the unnormalized softmax
partials (numerator [H,64,S] and denominator [H,S], fused via a
ones-column on the value matmul) over its shard for all heads, plus the
causal segment attention for ONE head (head index == core index; the
program is identical on every core, only the per-core input data
differs).  The ring-buffer write is a DRAM->DRAM copy of the
host-prescattered shard.  The host combines partials (gather +
normalize only).

Score matmuls run transposed (scoresT[t, q]) so that both GEMM operands
use their natural / host-pretransposed layouts and no on-chip
transposes are needed:
  scoresT = kT_tile.T @ qT          (fp16, contraction over D=64)
  p       = exp(scoresT)            (ScalarE, PSUM -> SBUF fp16)
  y_acc[q-chunk] += p_chunk.T @ [v_tile | 1]   (contraction over t=128)
The PV matmul keeps the LARGE operand (p, full 128 columns, fp16) as
the stationary side so the weight port's fast-weight-load (2 cols/cyc,
overlapped with the previous matmul's stream) absorbs it, and streams
only 65 columns of [v | ones]; column 64 of the accumulator is the
softmax denominator.  The PE clock on these parts is pinned at 1.2 GHz,
so minimizing total streamed matmul columns is what matters.
"""

import os

import numpy as np

os.environ.setdefault("MYCRO_LOCAL_CACHE", "1")

B, M, S, H, D = 1, 32768, 512, 8, 64
NCORES = 8
MC = M // NCORES  # memory rows per core
PAIRS = H // 2
NEG = np.float32(-1e30)
SCALE = np.float32(D) ** np.float32(-0.5)

_BUILD_CACHE = {}
QK_F16 = True


def _group_sizes(mt):
    """Split mt m-tiles into groups alternating between the two PSUM
    score slots (slot A: 4 banks, slot B: 3 banks; + accum = 8 banks).
    An EVEN number of groups per head keeps the A/B alternation intact
    across head boundaries so consecutive groups never share a slot
    (a shared slot serializes QK(g+1) behind exp(g) on the PE)."""
    gs = []
    rem = mt
    while rem >= 7:
        gs += [4, 3]
        rem -= 7
    if rem == 4:
        gs += [2, 2]
    elif rem == 3:
        gs += [2, 1]
    elif rem:
        gs += [rem]
    assert sum(gs) == mt
    return gs


def build_nc(mc=MC, s=S, h=H, d=D, use_f32r=True):
    """Build + compile the single-core Bass program (same on all cores)."""
    import concourse.bacc as bacc
    import concourse.mybir as mybir
    import concourse.tile as tile

    f32 = mybir.dt.float32
    bf16 = mybir.dt.float16
    mm_dt = mybir.dt.float16 if QK_F16 else (
        mybir.dt.float32r if use_f32r else f32)
    pairs = h // 2
    mt = mc // 128
    gsizes = _group_sizes(mt)
    st = s // 128  # segment m-tiles
    assert st <= 4

    nc = bacc.Bacc("TRN2", target_bir_lowering=False, debug=False,
                   num_devices=NCORES)

    kT_d = nc.dram_tensor("kt_sh", [pairs, 128, mc], mm_dt, kind="ExternalInput")
    vaug_d = nc.dram_tensor("memv_aug", [mc, h, d + 1], bf16, kind="ExternalInput")
    kscat_d = nc.dram_tensor("memk_scat", [mc, h, d], f32, kind="ExternalInput")
    vscat_d = nc.dram_tensor("memv_scat", [mc, h, d], f32, kind="ExternalInput")
    qT_d = nc.dram_tensor("qt_pack", [pairs, 128, s], mm_dt, kind="ExternalInput")
    ktseg_d = nc.dram_tensor("kt_seg", [d, s], mm_dt, kind="ExternalInput")
    qtseg_d = nc.dram_tensor("qt_seg", [d, s], mm_dt, kind="ExternalInput")
    vseg_d = nc.dram_tensor("v_seg", [s, d + 1], bf16, kind="ExternalInput")
    mask_d = nc.dram_tensor("maskt", [s, s], f32, kind="ExternalInput")

    nmk_d = nc.dram_tensor("new_memk", [mc, h, d], f32, kind="ExternalOutput")
    nmv_d = nc.dram_tensor("new_memv", [mc, h, d], f32, kind="ExternalOutput")
    part_d = nc.dram_tensor("mem_part", [h, 128, s // 128, d + 1], f32,
                            kind="ExternalOutput")
    seg_d = nc.dram_tensor("seg_part", [128, s // 128, d + 1], f32,
                           kind="ExternalOutput")

    Exp = mybir.ActivationFunctionType.Exp

    # DRAM->DRAM scatter copy chunks, paced across the kernel so they
    # never monopolize the DMA rings / HBM ahead of compute.
    d2d = []
    crows = max(128, mc // 16)
    for src, dst in ((kscat_d, nmk_d), (vscat_d, nmv_d)):
        for r0 in range(0, mc, crows):
            r1 = min(mc, r0 + crows)
            d2d.append((dst[r0:r1], src[r0:r1]))

    # per-group metadata for the software pipeline: QK(g)+exp(g) are
    # emitted one group AHEAD of PV(g-1) so the PE never waits on exp.
    # Score PSUM slot alternates by global parity (A: 4 banks, B: 3).
    groups = []
    for head in range(h):
        mt0 = 0
        for gs in gsizes:
            groups.append(dict(head=head, mt0=mt0, gs=gs, seg=False))
            mt0 += gs
    groups.append(dict(head=h, mt0=0, gs=st, seg=True))
    prev_slot = 1
    for g in groups:
        slot = 1 - prev_slot
        if g["gs"] > (4, 3)[slot]:
            slot = 0
        g["slot"] = slot
        prev_slot = slot
    d2d_every = max(1, len(groups) // (len(d2d) + 1))

    with tile.TileContext(nc) as tc:
        with (
            tc.tile_pool(name="big", bufs=1) as big,
            tc.tile_pool(name="vp", bufs=7) as vp,
            tc.tile_pool(name="pp", bufs=3) as pp,
            tc.tile_pool(name="op", bufs=2) as op,
            tc.tile_pool(name="ps", bufs=1, space="PSUM") as ps,
        ):
            # queries first (every QK reads them), then kT chunks drip-fed
            qt = big.tile([128, pairs, s], mm_dt, tag="qt")
            for p in range(pairs):
                nc.sync.dma_start(out=qt[:, p, :], in_=qT_d[p])

            nchunk = (mt + 3) // 4
            kt = [[None] * nchunk for _ in range(pairs)]
            kt_todo = [(p, c) for p in range(pairs) for c in range(nchunk)]

            def drip_kt(n):
                for _ in range(n):
                    if not kt_todo:
                        return
                    p, c = kt_todo.pop(0)
                    c0, c1 = c * 512, min(mc, (c + 1) * 512)
                    t = big.tile([128, c1 - c0], mm_dt, tag=f"kt{p}_{c}",
                                 name=f"kt{p}_{c}")
                    nc.sync.dma_start(out=t[:], in_=kT_d[p, :, c0:c1])
                    kt[p][c] = t

            seg_tiles = {}

            def load_seg():
                maskt = big.tile([128, st, s], f32, tag="mask", name="maskt")
                nc.sync.dma_start(
                    out=maskt[:],
                    in_=mask_d.rearrange("(j p) q -> p j q", p=128))
                ktseg = big.tile([d, s], mm_dt, tag="ktseg", name="ktseg")
                nc.sync.dma_start(out=ktseg[:], in_=ktseg_d[:])
                qtseg = big.tile([d, s], mm_dt, tag="qtseg", name="qtseg")
                nc.sync.dma_start(out=qtseg[:], in_=qtseg_d[:])
                vseg = big.tile([128, st, d + 1], bf16, tag="vseg",
                                name="vseg")
                nc.sync.dma_start(
                    out=vseg[:],
                    in_=vseg_d.rearrange("(j p) f -> p j f", p=128))
                seg_tiles.update(maskt=maskt, ktseg=ktseg, qtseg=qtseg,
                                 vseg=vseg)

            accs = {}
            n_pv = {}

            def emit_vt(g):
                """Prefetch the value tile for group g."""
                if g["seg"]:
                    g["vt"] = seg_tiles["vseg"]
                    return
                gs, mt0, head = g["gs"], g["mt0"], g["head"]
                vt = vp.tile([128, gs, d + 1], bf16, tag="vt", name="vt")
                nc.sync.dma_start(
                    out=vt[:],
                    in_=vaug_d[mt0 * 128:(mt0 + gs) * 128, head, :]
                    .rearrange("(j p) f -> p j f", p=128))
                g["vt"] = vt

            def emit_qk_exp(g):
                """QK matmuls + (mask) + exp for group g."""
                gs, mt0 = g["gs"], g["mt0"]
                cap = (4, 3)[g["slot"]]
                sc = ps.tile([128, cap, s], f32, tag=f"sc{g['slot']}",
                             name=f"sc{g['slot']}")
                pt = pp.tile([128, cap, s], bf16, tag=f"pt{g['slot']}",
                             name=f"pt{g['slot']}")
                if g["seg"]:
                    for j in range(gs):
                        t = mt0 + j
                        nc.tensor.matmul(
                            sc[:, j, :],
                            seg_tiles["ktseg"][:, t * 128:(t + 1) * 128],
                            seg_tiles["qtseg"][:], start=True, stop=True)
                    nc.vector.tensor_add(
                        sc[:, :gs, :], sc[:, :gs, :],
                        seg_tiles["maskt"][:, mt0:mt0 + gs, :])
                else:
                    head = g["head"]
                    pair, half = head // 2, head % 2
                    p0 = 64 * half
                    for j in range(gs):
                        m = mt0 + j
                        nc.tensor.matmul(
                            sc[:, j, :],
                            kt[pair][m // 4][p0:p0 + d,
                                             (m % 4) * 128:(m % 4) * 128 + 128],
                            qt[p0:p0 + d, pair, :],
                            start=True, stop=True)
                nc.scalar.activation(pt[:, :gs, :], sc[:, :gs, :], Exp)
                g["pt"] = pt

            nq = s // 128

            def emit_pv(g):
                """PV accumulation for group g (+ spill when head done).
                Flipped orientation: p chunk is the stationary operand
                (bf16 full-128-col -> FWL), v streams (65 cols/tile)."""
                head, gs = g["head"], g["gs"]
                last_n = st if g["seg"] else mt
                if head not in accs:
                    accs[head] = ps.tile([128, nq, d + 1], f32, tag="acc",
                                         name=f"acc{head}")
                    n_pv[head] = 0
                acc = accs[head]
                voff = g["mt0"] if g["seg"] else 0
                for j in range(gs):
                    for qc in range(nq):
                        # start only on the head's very first matmul:
                        # start=True marks the whole 2KB PSUM bank as
                        # pending-zero, which covers all nq accumulators.
                        nc.tensor.matmul(
                            acc[:, qc, :],
                            g["pt"][:, j, qc * 128:(qc + 1) * 128],
                            g["vt"][:, voff + j, :],
                            start=(n_pv[head] == 0 and qc == 0),
                            stop=(n_pv[head] == last_n - 1),
                            skip_group_check=True)
                    n_pv[head] += 1
                if n_pv[head] == last_n:
                    yt = op.tile([128, nq, d + 1], f32, tag="yt", name="yt")
                    nc.vector.tensor_copy(yt[:], acc[:])
                    dst = seg_d[:] if g["seg"] else part_d[head]
                    nc.sync.dma_start(out=dst, in_=yt[:])
                    del accs[head]

            ng = len(groups)
            seg_load_at = max(0, ng - 8)
            if seg_load_at == 0:
                load_seg()
            prev = None
            nd2d = 0
            for gi, g in enumerate(groups):
                if gi == 0:
                    for gg in groups[:3]:
                        if not gg["seg"]:
                            emit_vt(gg)
                elif gi + 2 < ng and not groups[gi + 2]["seg"]:
                    emit_vt(groups[gi + 2])
                drip_kt(2)
                if gi == seg_load_at and gi > 0:
                    load_seg()
                if g["seg"]:
                    emit_vt(g)
                emit_qk_exp(g)
                if prev is not None:
                    emit_pv(prev)
                prev = g
                if gi % d2d_every == d2d_every - 1 and nd2d < len(d2d):
                    dst, src = d2d[nd2d]
                    nc.sync.dma_start(out=dst, in_=src)
                    nd2d += 1
            emit_pv(prev)
            drip_kt(len(kt_todo))
            while nd2d < len(d2d):
                dst, src = d2d[nd2d]
                nc.sync.dma_start(out=dst, in_=src)
                nd2d += 1

    nc.compile()
    return nc


def _get_nc():
    key = (MC, S, H, D)
    if key not in _BUILD_CACHE:
        _BUILD_CACHE[key] = build_nc()
    return _BUILD_CACHE[key]


def make_in_maps(mem_keys, mem_vals, keys, values, queries,
                 start_of_sequence, write_index,
                 mc=MC, ncores=NCORES):
    """Host-side input marshaling: clearing, prescatter, shard, transpose."""
    mem_keys = np.asarray(mem_keys, np.float32)
    mem_vals = np.asarray(mem_vals, np.float32)
    keys = np.asarray(keys, np.float32)
    values = np.asarray(values, np.float32)
    queries = np.asarray(queries, np.float32)
    m = mem_keys.shape[1]
    s, h, d = keys.shape[1:]
    pairs = h // 2

    keep = not bool(np.asarray(start_of_sequence).reshape(-1)[0])
    mk = mem_keys[0] if keep else np.zeros_like(mem_keys[0])
    mv = mem_vals[0] if keep else np.zeros_like(mem_vals[0])
    k_seg, v_seg = keys[0], values[0]
    q = queries[0] * np.float32(d) ** np.float32(-0.5)

    wi = int(np.asarray(write_index))
    wi_c = max(0, min(wi, m - s))
    k_scat = mk.copy()
    k_scat[wi_c:wi_c + s] = k_seg
    v_scat = mv.copy()
    v_scat[wi_c:wi_c + s] = v_seg

    bf16 = np.float16
    kq_dt = np.float16 if QK_F16 else np.float32

    tri = np.arange(s, dtype=np.int32)
    maskT = np.where(tri[:, None] < tri[None, :],
                     np.float32(0.0), NEG).astype(np.float32)
    qT = np.ascontiguousarray(q.transpose(1, 2, 0).reshape(pairs, 2 * d, s))
    ones_mc = np.ones((mc, h, 1), np.float32)
    ones_s = np.ones((s, 1), np.float32)

    in_maps = []
    for c in range(ncores):
        r0, r1 = c * mc, (c + 1) * mc
        in_maps.append({
            "kt_sh": np.ascontiguousarray(
                mk[r0:r1].transpose(1, 2, 0)
                .reshape(pairs, 2 * d, mc)).astype(kq_dt),
            "memv_aug": np.concatenate(
                [mv[r0:r1], ones_mc], axis=2).astype(bf16),
            "memk_scat": np.ascontiguousarray(k_scat[r0:r1]),
            "memv_scat": np.ascontiguousarray(v_scat[r0:r1]),
            "qt_pack": qT.astype(kq_dt),
            "kt_seg": np.ascontiguousarray(k_seg[:, c % h, :].T).astype(kq_dt),
            "qt_seg": np.ascontiguousarray(q[:, c % h, :].T).astype(kq_dt),
            "v_seg": np.concatenate(
                [v_seg[:, c % h, :], ones_s], axis=1).astype(bf16),
            "maskt": maskT,
        })
    return in_maps


def combine(results, m, s, h, d, wi, ncores=NCORES):
    """Gather per-core partials into the final outputs."""
    new_mk = np.concatenate(
        [results[c]["new_memk"] for c in range(ncores)], axis=0)[None]
    new_mv = np.concatenate(
        [results[c]["new_memv"] for c in range(ncores)], axis=0)[None]
    tot = np.zeros((h, 128, s // 128, d + 1), np.float32)
    for c in range(ncores):
        tot += results[c]["mem_part"]
    for head in range(h):
        tot[head] += results[head % ncores]["seg_part"]
    # tot[h, r, qc, :]: query index = qc*128 + r
    y = (tot[..., :d] / tot[..., d:d + 1])  # [h, r, qc, d]
    y = y.transpose(2, 1, 0, 3).reshape(s, h, d)[None]
    new_wi = np.int32((wi + s) % m)
    return (np.ascontiguousarray(y, dtype=np.float32), new_mk, new_mv, new_wi)


def kernel(mem_keys, mem_vals, keys, values, queries,
           start_of_sequence, write_index):
    from concourse.bass_utils import run_bass_kernel_spmd

    in_maps = make_in_maps(mem_keys, mem_vals, keys, values, queries,
                           start_of_sequence, write_index)
    nc = _get_nc()
    res = run_bass_kernel_spmd(nc, in_maps, core_ids=list(range(NCORES)))
    kernel._last_results = res
    wi = int(np.asarray(write_index))
    return combine(res.results, M, S, H, D, wi)


# revision 21
# speedup vs baseline: 1.3760x; 1.0934x over previous
"""DenseKVMemory Trainium2 kernel.

Reference op (B=1, M=32768, S=512, H=8, D=64, fp32):
  - clear memory at doc boundaries (start_of_sequence)
  - joint attention over [memory (unmasked), segment (strict causal)]
  - scatter-write segment K/V into the ring buffer at write_index

Distribution: memory rows are sharded evenly across the 8 cores (4096
rows/core, all heads).  Each core computes the unnormalized softmax
partials (numerator [H,64,S] and denominator [H,S], fused via a
ones-column on the value matmul) over its shard for all heads, plus the
causal segment attention for ONE head (head index == core index; the
program is identical on every core, only the per-core input data
differs).  The ring-buffer write is a DRAM->DRAM copy of the
host-prescattered shard.  The host combines partials (gather +
normalize only).

Score matmuls run transposed (scoresT[t, q]) so that both GEMM operands
use their natural / host-pretransposed layouts and no on-chip
transposes are needed:
  scoresT = kT_tile.T @ qT          (fp16, contraction over D=64)
  p       = exp(scoresT)            (ScalarE, PSUM -> SBUF fp16)
  y_acc[q-chunk] += p_chunk.T @ [v_tile | 1]   (contraction over t=128)
The PV matmul keeps the LARGE operand (p, full 128 columns, fp16) as
the stationary side so the weight port's fast-weight-load (2 cols/cyc,
overlapped with the previous matmul's stream) absorbs it, and streams
only 65 columns of [v | ones]; column 64 of the accumulator is the
softmax denominator.  The PE clock on these parts is pinned at 1.2 GHz,
so minimizing total streamed matmul columns is what matters.
"""

import os

import numpy as np

os.environ.setdefault("MYCRO_LOCAL_CACHE", "1")

B, M, S, H, D = 1, 32768, 512, 8, 64
NCORES = 8
MC = M // NCORES  # memory rows per core
PAIRS = H // 2
NEG = np.float32(-1e30)
SCALE = np.float32(D) ** np.float32(-0.5)

_BUILD_CACHE = {}
QK_F16 = True


def _group_sizes(mt):
    """Split mt m-tiles into groups alternating between the two PSUM
    score slots (slot A: 4 banks, slot B: 3 banks; + accum = 8 banks).
    An EVEN number of groups per head keeps the A/B alternation intact
    across head boundaries so consecutive groups never share a slot
    (a shared slot serializes QK(g+1) behind exp(g) on the PE)."""
    gs = []
    rem = mt
    while rem >= 7:
        gs += [4, 3]
        rem -= 7
    if rem == 4:
        gs += [2, 2]
    elif rem == 3:
        gs += [2, 1]
    elif rem:
        gs += [rem]
    assert sum(gs) == mt
    return gs


def build_nc(mc=MC, s=S, h=H, d=D, use_f32r=True):
    """Build + compile the single-core Bass program (same on all cores)."""
    import concourse.bacc as bacc
    import concourse.mybir as mybir
    import concourse.tile as tile

    f32 = mybir.dt.float32
    bf16 = mybir.dt.float16
    mm_dt = mybir.dt.float16 if QK_F16 else (
        mybir.dt.float32r if use_f32r else f32)
    pairs = h // 2
    mt = mc // 128
    gsizes = _group_sizes(mt)
    st = s // 128  # segment m-tiles
    assert st <= 4

    nc = bacc.Bacc("TRN2", target_bir_lowering=False, debug=False,
                   num_devices=NCORES)

    kT_d = nc.dram_tensor("kt_sh", [pairs, 128, mc], mm_dt, kind="ExternalInput")
    vaug_d = nc.dram_tensor("memv_aug", [mc, h, d + 1], bf16, kind="ExternalInput")
    kscat_d = nc.dram_tensor("memk_scat", [mc, h, d], f32, kind="ExternalInput")
    vscat_d = nc.dram_tensor("memv_scat", [mc, h, d], f32, kind="ExternalInput")
    qT_d = nc.dram_tensor("qt_pack", [pairs, 128, s], mm_dt, kind="ExternalInput")
    ktseg_d = nc.dram_tensor("kt_seg", [d, s], mm_dt, kind="ExternalInput")
    qtseg_d = nc.dram_tensor("qt_seg", [d, s], mm_dt, kind="ExternalInput")
    vseg_d = nc.dram_tensor("v_seg", [s, d + 1], bf16, kind="ExternalInput")
    mask_d = nc.dram_tensor("maskt", [s, s], f32, kind="ExternalInput")

    nmk_d = nc.dram_tensor("new_memk", [mc, h, d], f32, kind="ExternalOutput")
    nmv_d = nc.dram_tensor("new_memv", [mc, h, d], f32, kind="ExternalOutput")
    part_d = nc.dram_tensor("mem_part", [h, 128, s // 128, d + 1], f32,
                            kind="ExternalOutput")
    seg_d = nc.dram_tensor("seg_part", [128, s // 128, d + 1], f32,
                           kind="ExternalOutput")

    Exp = mybir.ActivationFunctionType.Exp

    # DRAM->DRAM scatter copy chunks, paced across the kernel so they
    # never monopolize the DMA rings / HBM ahead of compute.
    d2d = []
    crows = max(128, mc // 16)
    for src, dst in ((kscat_d, nmk_d), (vscat_d, nmv_d)):
        for r0 in range(0, mc, crows):
            r1 = min(mc, r0 + crows)
            d2d.append((dst[r0:r1], src[r0:r1]))

    # per-group metadata for the software pipeline: QK(g)+exp(g) are
    # emitted one group AHEAD of PV(g-1) so the PE never waits on exp.
    # Score PSUM slot alternates by global parity (A: 4 banks, B: 3).
    groups = []
    for head in range(h):
        mt0 = 0
        for gs in gsizes:
            groups.append(dict(head=head, mt0=mt0, gs=gs, seg=False))
            mt0 += gs
    groups.append(dict(head=h, mt0=0, gs=st, seg=True))
    prev_slot = 1
    for g in groups:
        slot = 1 - prev_slot
        if g["gs"] > (4, 3)[slot]:
            slot = 0
        g["slot"] = slot
        prev_slot = slot
    d2d_every = max(1, len(groups) // (len(d2d) + 1))

    with tile.TileContext(nc) as tc:
        with (
            tc.tile_pool(name="big", bufs=1) as big,
            tc.tile_pool(name="vp", bufs=7) as vp,
            tc.tile_pool(name="pp", bufs=3) as pp,
            tc.tile_pool(name="op", bufs=2) as op,
            tc.tile_pool(name="ps", bufs=1, space="PSUM") as ps,
        ):
            # queries first (every QK reads them), then kT chunks drip-fed
            qt = []
            for p in range(pairs):
                t = big.tile([128, s], mm_dt, tag=f"qt{p}", name=f"qt{p}")
                nc.sync.dma_start(out=t[:], in_=qT_d[p])
                qt.append(t)

            nchunk = (mt + 3) // 4
            kt = [[None] * nchunk for _ in range(pairs)]
            kt_todo = [(p, c) for p in range(pairs) for c in range(nchunk)]

            def drip_kt(n):
                for _ in range(n):
                    if not kt_todo:
                        return
                    p, c = kt_todo.pop(0)
                    c0, c1 = c * 512, min(mc, (c + 1) * 512)
                    t = big.tile([128, c1 - c0], mm_dt, tag=f"kt{p}_{c}",
                                 name=f"kt{p}_{c}")
                    nc.sync.dma_start(out=t[:], in_=kT_d[p, :, c0:c1])
                    kt[p][c] = t

            seg_tiles = {}

            def load_seg():
                maskt = big.tile([128, st, s], f32, tag="mask", name="maskt")
                nc.sync.dma_start(
                    out=maskt[:],
                    in_=mask_d.rearrange("(j p) q -> p j q", p=128))
                ktseg = big.tile([d, s], mm_dt, tag="ktseg", name="ktseg")
                nc.sync.dma_start(out=ktseg[:], in_=ktseg_d[:])
                qtseg = big.tile([d, s], mm_dt, tag="qtseg", name="qtseg")
                nc.sync.dma_start(out=qtseg[:], in_=qtseg_d[:])
                vseg = big.tile([128, st, d + 1], bf16, tag="vseg",
                                name="vseg")
                nc.sync.dma_start(
                    out=vseg[:],
                    in_=vseg_d.rearrange("(j p) f -> p j f", p=128))
                seg_tiles.update(maskt=maskt, ktseg=ktseg, qtseg=qtseg,
                                 vseg=vseg)

            accs = {}
            n_pv = {}

            def emit_vt(g):
                """Prefetch the value tile for group g."""
                if g["seg"]:
                    g["vt"] = seg_tiles["vseg"]
                    return
                gs, mt0, head = g["gs"], g["mt0"], g["head"]
                vt = vp.tile([128, gs, d + 1], bf16, tag="vt", name="vt")
                nc.sync.dma_start(
                    out=vt[:],
                    in_=vaug_d[mt0 * 128:(mt0 + gs) * 128, head, :]
                    .rearrange("(j p) f -> p j f", p=128))
                g["vt"] = vt

            def emit_qk_exp(g):
                """QK matmuls + (mask) + exp for group g."""
                gs, mt0 = g["gs"], g["mt0"]
                cap = (4, 3)[g["slot"]]
                sc = ps.tile([128, cap, s], f32, tag=f"sc{g['slot']}",
                             name=f"sc{g['slot']}")
                pt = pp.tile([128, cap, s], bf16, tag=f"pt{g['slot']}",
                             name=f"pt{g['slot']}")
                if g["seg"]:
                    for j in range(gs):
                        t = mt0 + j
                        nc.tensor.matmul(
                            sc[:, j, :],
                            seg_tiles["ktseg"][:, t * 128:(t + 1) * 128],
                            seg_tiles["qtseg"][:], start=True, stop=True)
                    nc.vector.tensor_add(
                        sc[:, :gs, :], sc[:, :gs, :],
                        seg_tiles["maskt"][:, mt0:mt0 + gs, :])
                else:
                    head = g["head"]
                    pair, half = head // 2, head % 2
                    p0 = 64 * half
                    for j in range(gs):
                        m = mt0 + j
                        nc.tensor.matmul(
                            sc[:, j, :],
                            kt[pair][m // 4][p0:p0 + d,
                                             (m % 4) * 128:(m % 4) * 128 + 128],
                            qt[pair][p0:p0 + d, :],
                            start=True, stop=True)
                nc.scalar.activation(pt[:, :gs, :], sc[:, :gs, :], Exp)
                g["pt"] = pt

            nq = s // 128

            def emit_pv(g):
                """PV accumulation for group g (+ spill when head done).
                Flipped orientation: p chunk is the stationary operand
                (bf16 full-128-col -> FWL), v streams (65 cols/tile)."""
                head, gs = g["head"], g["gs"]
                last_n = st if g["seg"] else mt
                if head not in accs:
                    accs[head] = ps.tile([128, nq, d + 1], f32, tag="acc",
                                         name=f"acc{head}")
                    n_pv[head] = 0
                acc = accs[head]
                voff = g["mt0"] if g["seg"] else 0
                for j in range(gs):
                    for qc in range(nq):
                        # start only on the head's very first matmul:
                        # start=True marks the whole 2KB PSUM bank as
                        # pending-zero, which covers all nq accumulators.
                        nc.tensor.matmul(
                            acc[:, qc, :],
                            g["pt"][:, j, qc * 128:(qc + 1) * 128],
                            g["vt"][:, voff + j, :],
                            start=(n_pv[head] == 0 and qc == 0),
                            stop=(n_pv[head] == last_n - 1),
                            skip_group_check=True)
                    n_pv[head] += 1
                if n_pv[head] == last_n:
                    yt = op.tile([128, nq, d + 1], f32, tag="yt", name="yt")
                    nc.vector.tensor_copy(yt[:], acc[:])
                    dst = seg_d[:] if g["seg"] else part_d[head]
                    nc.sync.dma_start(out=dst, in_=yt[:])
                    del accs[head]

            ng = len(groups)
            seg_load_at = max(0, ng - 8)
            if seg_load_at == 0:
                load_seg()
            prev = None
            nd2d = 0
            for gi, g in enumerate(groups):
                if gi == 0:
                    for gg in groups[:3]:
                        if not gg["seg"]:
                            emit_vt(gg)
                elif gi + 2 < ng and not groups[gi + 2]["seg"]:
                    emit_vt(groups[gi + 2])
                drip_kt(2)
                if gi == seg_load_at and gi > 0:
                    load_seg()
                if g["seg"]:
                    emit_vt(g)
                emit_qk_exp(g)
                if prev is not None:
                    emit_pv(prev)
                prev = g
                if gi % d2d_every == d2d_every - 1 and nd2d < len(d2d):
                    dst, src = d2d[nd2d]
                    nc.scalar.dma_start(out=dst, in_=src)
                    nd2d += 1
            emit_pv(prev)
            drip_kt(len(kt_todo))
            while nd2d < len(d2d):
                dst, src = d2d[nd2d]
                nc.scalar.dma_start(out=dst, in_=src)
                nd2d += 1

    nc.compile()
    return nc


def _get_nc():
    key = (MC, S, H, D)
    if key not in _BUILD_CACHE:
        _BUILD_CACHE[key] = build_nc()
    return _BUILD_CACHE[key]


def make_in_maps(mem_keys, mem_vals, keys, values, queries,
                 start_of_sequence, write_index,
                 mc=MC, ncores=NCORES):
    """Host-side input marshaling: clearing, prescatter, shard, transpose."""
    mem_keys = np.asarray(mem_keys, np.float32)
    mem_vals = np.asarray(mem_vals, np.float32)
    keys = np.asarray(keys, np.float32)
    values = np.asarray(values, np.float32)
    queries = np.asarray(queries, np.float32)
    m = mem_keys.shape[1]
    s, h, d = keys.shape[1:]
    pairs = h // 2

    keep = not bool(np.asarray(start_of_sequence).reshape(-1)[0])
    mk = mem_keys[0] if keep else np.zeros_like(mem_keys[0])
    mv = mem_vals[0] if keep else np.zeros_like(mem_vals[0])
    k_seg, v_seg = keys[0], values[0]
    q = queries[0] * np.float32(d) ** np.float32(-0.5)

    wi = int(np.asarray(write_index))
    wi_c = max(0, min(wi, m - s))
    k_scat = mk.copy()
    k_scat[wi_c:wi_c + s] = k_seg
    v_scat = mv.copy()
    v_scat[wi_c:wi_c + s] = v_seg

    bf16 = np.float16
    kq_dt = np.float16 if QK_F16 else np.float32

    tri = np.arange(s, dtype=np.int32)
    maskT = np.where(tri[:, None] < tri[None, :],
                     np.float32(0.0), NEG).astype(np.float32)
    qT = np.ascontiguousarray(q.transpose(1, 2, 0).reshape(pairs, 2 * d, s))
    ones_mc = np.ones((mc, h, 1), np.float32)
    ones_s = np.ones((s, 1), np.float32)

    in_maps = []
    for c in range(ncores):
        r0, r1 = c * mc, (c + 1) * mc
        in_maps.append({
            "kt_sh": np.ascontiguousarray(
                mk[r0:r1].transpose(1, 2, 0)
                .reshape(pairs, 2 * d, mc)).astype(kq_dt),
            "memv_aug": np.concatenate(
                [mv[r0:r1], ones_mc], axis=2).astype(bf16),
            "memk_scat": np.ascontiguousarray(k_scat[r0:r1]),
            "memv_scat": np.ascontiguousarray(v_scat[r0:r1]),
            "qt_pack": qT.astype(kq_dt),
            "kt_seg": np.ascontiguousarray(k_seg[:, c % h, :].T).astype(kq_dt),
            "qt_seg": np.ascontiguousarray(q[:, c % h, :].T).astype(kq_dt),
            "v_seg": np.concatenate(
                [v_seg[:, c % h, :], ones_s], axis=1).astype(bf16),
            "maskt": maskT,
        })
    return in_maps


def combine(results, m, s, h, d, wi, ncores=NCORES):
    """Gather per-core partials into the final outputs."""
    new_mk = np.concatenate(
        [results[c]["new_memk"] for c in range(ncores)], axis=0)[None]
    new_mv = np.concatenate(
        [results[c]["new_memv"] for c in range(ncores)], axis=0)[None]
    tot = np.zeros((h, 128, s // 128, d + 1), np.float32)
    for c in range(ncores):
        tot += results[c]["mem_part"]
    for head in range(h):
        tot[head] += results[head % ncores]["seg_part"]
    # tot[h, r, qc, :]: query index = qc*128 + r
    y = (tot[..., :d] / tot[..., d:d + 1])  # [h, r, qc, d]
    y = y.transpose(2, 1, 0, 3).reshape(s, h, d)[None]
    new_wi = np.int32((wi + s) % m)
    return (np.ascontiguousarray(y, dtype=np.float32), new_mk, new_mv, new_wi)


def kernel(mem_keys, mem_vals, keys, values, queries,
           start_of_sequence, write_index):
    from concourse.bass_utils import run_bass_kernel_spmd

    in_maps = make_in_maps(mem_keys, mem_vals, keys, values, queries,
                           start_of_sequence, write_index)
    nc = _get_nc()
    res = run_bass_kernel_spmd(nc, in_maps, core_ids=list(range(NCORES)))
    kernel._last_results = res
    wi = int(np.asarray(write_index))
    return combine(res.results, M, S, H, D, wi)
